# revision 1
# baseline (speedup 1.0000x reference)
"""Chunked attention kernel for Trainium2 (Bass/Tile), SPMD over 8 NeuronCores.

Problem (hardcoded):
  x: [B=8, C=1024, L=4096] fp32, Wq/Wk/Wv/Wo: [1024,1024] fp32 (stored [in,out]),
  biases [1024] fp32.  H=8 heads, head_dim=128, CHUNK=64 (block-diagonal attention).
  out = transpose(softmax((xt@Wq)(xt@Wk)^T/sqrt(128) blockwise) @ (xt@Wv) @ Wo, [B,C,L])

Sharding: data-parallel over B — one batch per core. No collectives.

Per-core dataflow (all matmuls fp16 in / fp32 PSUM accumulate):
  Q^T[c,l] = matmul(lhsT=Wq, rhs=x)        (feature-major, x arrives [C,L] = ready)
  K^T[c,l] = matmul(lhsT=Wk, rhs=x)
  V[l,c]   = matmul(lhsT=x,  rhs=Wv)       (token-major)
  per head h, chunk-pair p (128 tokens):
    S^T[k,q] = matmul(lhsT=K^T block, rhs=Q^T block)   (scores transposed)
    E = exp(S^T/sqrt(128)) on the two diagonal 64x64 blocks (ACT), rest zero
    D = matmul(lhsT=ones[128,128], rhs=E)  -> denominator replicated on all partitions
    R = 1/D (DVE reciprocal), EN = E*R     (normalized attn, transposed)
    P^T[d,q] = matmul(lhsT=V block, rhs=EN)
  out^T[c,l] = matmul(lhsT=Wo, rhs=P^T)    -> exactly the [C,L] output layout
"""

import numpy as np
from contextlib import ExitStack

import concourse.bass as bass
import concourse.bacc as bacc
import concourse.tile as tile
import concourse.mybir as mybir

B, C, L = 8, 1024, 4096
H, HD, CHUNK, PAIR = 8, 128, 64, 128
N_CORES = 8
KT = C // 128          # 8 contraction tiles
LT = 512               # tokens per strip
F16 = mybir.dt.float16
F32 = mybir.dt.float32
SCALE = 1.0 / float(np.sqrt(HD))
WNAMES = ("wq", "wk", "wv", "wo")


def _emit(ctx, tc, x_d, w_d, o_d, l_total):
    nc = tc.nc
    NS = l_total // LT     # strips
    NP = LT // PAIR        # chunk-pairs (= token 128-tiles) per strip

    wpool = ctx.enter_context(tc.tile_pool(name="w", bufs=1))
    cpool = ctx.enter_context(tc.tile_pool(name="const", bufs=1))
    xpool = ctx.enter_context(tc.tile_pool(name="xp", bufs=2))
    qpool = ctx.enter_context(tc.tile_pool(name="qp", bufs=2))
    vpool = ctx.enter_context(tc.tile_pool(name="vp", bufs=2))
    epool = ctx.enter_context(tc.tile_pool(name="ep", bufs=1))
    rpool = ctx.enter_context(tc.tile_pool(name="rp", bufs=1))
    npool = ctx.enter_context(tc.tile_pool(name="np", bufs=2))
    ppool = ctx.enter_context(tc.tile_pool(name="pp", bufs=2))
    opool = ctx.enter_context(tc.tile_pool(name="op", bufs=1))
    pjps = ctx.enter_context(tc.tile_pool(name="pj", bufs=4, space="PSUM"))
    scps = ctx.enter_context(tc.tile_pool(name="sc", bufs=2, space="PSUM"))
    pvps = ctx.enter_context(tc.tile_pool(name="pv", bufs=2, space="PSUM"))

    def o_proj(p_t, ls):
        o_t = opool.tile([128, KT * LT], F32, tag="o")
        for m in range(KT):
            ps = pjps.tile([128, 512], F32, tag="pj")
            for j in range(KT):
                nc.tensor.matmul(ps[:, 0:LT],
                                 wt[("wo", j)][:, m * 128:(m + 1) * 128],
                                 p_t[:, j * LT:(j + 1) * LT],
                                 start=(j == 0), stop=(j == KT - 1))
            nc.vector.tensor_copy(o_t[:, m * LT:(m + 1) * LT], ps[:, 0:LT])
            nc.sync.dma_start(o_d[m * 128:(m + 1) * 128, ls:ls + LT],
                              o_t[:, m * LT:(m + 1) * LT])

    def load_x(s):
        x_t = xpool.tile([128, KT * LT], F16, tag="x")
        for j in range(KT):
            nc.sync.dma_start(x_t[:, j * LT:(j + 1) * LT],
                              x_d[j * 128:(j + 1) * 128, s * LT:(s + 1) * LT])
        return x_t

    # Startup DMA order follows first-use order: strip-0 x and wq k-tiles
    # interleaved (each Q-proj matmul waits only its own k-tile), then wk
    # (first K-proj), then wv/wo.
    wt = {}

    def load_w(n, j):
        t = wpool.tile([128, C], F16, tag=f"{n}{j}")
        nc.sync.dma_start(t[:], w_d[n][j * 128:(j + 1) * 128, :])
        wt[(n, j)] = t

    x_t0 = xpool.tile([128, KT * LT], F16, tag="x")
    for j in range(KT):
        nc.sync.dma_start(x_t0[:, j * LT:(j + 1) * LT],
                          x_d[j * 128:(j + 1) * 128, 0:LT])
        load_w("wq", j)
    for j in range(KT):
        load_w("wk", j)
    for j in range(KT):
        load_w("wv", j)
    for j in range(KT):
        load_w("wo", j)
    x_next = x_t0
    ones = cpool.tile([128, PAIR], F16, tag="ones")
    nc.vector.memset(ones[:], 1.0)
    # e_t is a single persistent buffer: exps rewrite the diagonal blocks every
    # strip, the off-diagonal stays zero from this one memset.
    e_t = epool.tile([128, H * LT], F16, tag="e")
    nc.gpsimd.memset(e_t[:], 0.0)

    for s in range(NS):
        ls = s * LT
        x_t = x_next if s == 0 else load_x(s)

        qk_t = qpool.tile([128, 2 * KT * LT], F16, tag="qk")
        r_t = rpool.tile([128, H * LT], F16, tag="r")
        en_t = npool.tile([128, H * LT], F16, tag="en")

        def denom_group(g):
            ps = pjps.tile([128, 512], F32, tag="pj")
            nc.tensor.matmul(ps[:], ones[:], e_t[:, g * 512:(g + 1) * 512],
                             start=True, stop=True)
            with nc.allow_low_precision(reason="softmax recip fp16 ample"):
                nc.vector.reciprocal(r_t[:, g * 512:(g + 1) * 512], ps[:])
            nc.vector.tensor_mul(en_t[:, g * 512:(g + 1) * 512],
                                 e_t[:, g * 512:(g + 1) * 512],
                                 r_t[:, g * 512:(g + 1) * 512])

        # --- Q/K projections interleaved with per-head score matmuls (keeps
        # --- the ACT exp drain spread across the strip instead of bunched)
        for h in range(H):
            qb = h * 2 * LT           # Q cols for head h
            kb = h * 2 * LT + LT      # K cols for head h
            for off, nm in ((qb, "wq"), (kb, "wk")):
                ps = pjps.tile([128, 512], F32, tag="pj")
                for j in range(KT):
                    nc.tensor.matmul(ps[:, 0:LT],
                                     wt[(nm, j)][:, h * 128:(h + 1) * 128],
                                     x_t[:, j * LT:(j + 1) * LT],
                                     start=(j == 0), stop=(j == KT - 1))
                nc.vector.tensor_copy(qk_t[:, off:off + LT], ps[:, 0:LT])
            sc = scps.tile([128, LT], F32, tag="sc")
            for p in range(NP):
                nc.tensor.matmul(sc[:, p * PAIR:(p + 1) * PAIR],
                                 qk_t[:, kb + p * PAIR:kb + (p + 1) * PAIR],
                                 qk_t[:, qb + p * PAIR:qb + (p + 1) * PAIR],
                                 start=True, stop=True)
            # exp of the diagonal 64x64 blocks of every pair -> e_t (off-diag
            # stays 0). One strided ACT per half: [64, (pairs), 64] pattern.
            eh = e_t[:, h * LT:(h + 1) * LT]
            for r0, c0 in ((0, 0), (64, 64)):
                nc.scalar.activation(
                    eh[r0:r0 + 64, :].rearrange("a (np c) -> a np c", c=PAIR)[:, :, c0:c0 + 64],
                    sc[r0:r0 + 64, :].rearrange("a (np c) -> a np c", c=PAIR)[:, :, c0:c0 + 64],
                    mybir.ActivationFunctionType.Exp, scale=SCALE)
            # softmax denominators, one 512-col group (= 2 heads) at a time:
            # ones-matmul (colsum replicated on all partitions) -> reciprocal ->
            # normalize. Group g covers heads 2g,2g+1; emitted at head 2g+3 so
            # ~2 heads of projection matmuls sit between the exp ACTs and the
            # colsum matmul that waits on them (PE is in-order).
            ng = (H * LT) // 512
            gph = ng // H              # denom groups per head (1 at LT=512)
            if gph == 1 and h >= 3:
                denom_group(h - 3)
            elif gph == 0 and h in (3, 5):
                denom_group((h - 3) // 2)

        # --- V projection (token-major): V[l, c] per 128-token tile
        v_t = vpool.tile([128, NP * C], F16, tag="v")
        ng = (H * LT) // 512
        done = (H - 3) if (ng // H) == 1 else 2
        for p in range(NP):
            if p == 1:
                for g in range(done, ng):
                    denom_group(g)
            for n2 in range(C // 512):
                ps = pjps.tile([128, 512], F32, tag="pj")
                for j in range(KT):
                    nc.tensor.matmul(ps[:],
                                     x_t[:, j * LT + p * 128:j * LT + (p + 1) * 128],
                                     wt[("wv", j)][:, n2 * 512:(n2 + 1) * 512],
                                     start=(j == 0), stop=(j == KT - 1))
                nc.vector.tensor_copy(v_t[:, p * C + n2 * 512:p * C + (n2 + 1) * 512],
                                      ps[:])

        # --- attention output: P^T[d, q], one merged psum + one ACT evict per head
        p_t = ppool.tile([128, KT * LT], F16, tag="p")
        for h in range(H):
            ps = pvps.tile([128, NP * PAIR], F32, tag="pv")
            for p in range(NP):
                nc.tensor.matmul(ps[:, p * PAIR:(p + 1) * PAIR],
                                 v_t[:, p * C + h * 128:p * C + (h + 1) * 128],
                                 en_t[:, h * LT + p * PAIR:h * LT + (p + 1) * PAIR],
                                 start=True, stop=True)
            nc.vector.tensor_copy(p_t[:, h * LT:(h + 1) * LT], ps[:])

        # --- output projection of the PREVIOUS strip (software pipelining: the
        # 6.8us of O-proj matmuls cover this strip's PV evictions and the next
        # strip's exp latency; O(s) only needs p_t(s), alive via ppool bufs=2)
        if s >= 1:
            o_proj(p_prev, ls_prev)
        p_prev, ls_prev = p_t, ls
    o_proj(p_prev, ls_prev)


def build_nc(l_total=L):
    nc = bacc.Bacc("TRN2", target_bir_lowering=False, debug=False,
                   enable_asserts=False)
    x_d = nc.dram_tensor("x", [C, l_total], F16, kind="ExternalInput").ap()
    w_d = {n: nc.dram_tensor(n, [C, C], F16, kind="ExternalInput").ap()
           for n in WNAMES}
    o_d = nc.dram_tensor("out", [C, l_total], F32, kind="ExternalOutput").ap()
    with tile.TileContext(nc) as tc:
        with ExitStack() as ctx:
            _emit(ctx, tc, x_d, w_d, o_d, l_total)
    nc.compile()
    return nc


_NC_CACHE = {}


def _get_nc(l_total):
    if l_total not in _NC_CACHE:
        _NC_CACHE[l_total] = build_nc(l_total)
    return _NC_CACHE[l_total]


def make_in_maps(x, Wq, Wk, Wv, Wo):
    x16 = np.ascontiguousarray(np.asarray(x).astype(np.float16))
    ws = {n: np.ascontiguousarray(np.asarray(w).astype(np.float16))
          for n, w in zip(WNAMES, (Wq, Wk, Wv, Wo))}
    in_maps = []
    for i in range(x.shape[0]):
        m = {"x": x16[i]}
        m.update(ws)
        in_maps.append(m)
    return in_maps


def _numpy_fallback(x, Wq, bq, Wk, bk, Wv, bv, Wo, bo):
    # Exact host-side path, used only if biases are nonzero (the problem spec
    # fills them with zeros, so the device kernel does not apply them).
    x = np.asarray(x, np.float32)
    Bn, Cn, Ln = x.shape
    hd = Cn // H
    nch = Ln // CHUNK
    xt = np.transpose(x, (0, 2, 1))
    Q = (xt @ Wq + bq).reshape(Bn, nch, CHUNK, H, hd)
    K = (xt @ Wk + bk).reshape(Bn, nch, CHUNK, H, hd)
    V = (xt @ Wv + bv).reshape(Bn, nch, CHUNK, H, hd)
    scores = np.einsum("bnqhd,bnkhd->bnhqk", Q, K) / np.sqrt(hd)
    scores -= scores.max(axis=-1, keepdims=True)
    e = np.exp(scores)
    attn = e / e.sum(axis=-1, keepdims=True)
    out = np.einsum("bnhqk,bnkhd->bnqhd", attn, V).reshape(Bn, Ln, Cn)
    out = out @ Wo + bo
    return np.ascontiguousarray(np.transpose(out, (0, 2, 1)).astype(np.float32))


def kernel(x, Wq, bq, Wk, bk, Wv, bv, Wo, bo, trace=False):
    from concourse.bass_utils import run_bass_kernel_spmd
    nb, c_in, l_total = x.shape
    if (any(np.any(np.asarray(b) != 0) for b in (bq, bk, bv, bo))
            or c_in != C or l_total % LT != 0 or nb > N_CORES):
        return _numpy_fallback(x, Wq, bq, Wk, bk, Wv, bv, Wo, bo)
    nc = _get_nc(l_total)
    in_maps = make_in_maps(x, Wq, Wk, Wv, Wo)
    res = run_bass_kernel_spmd(nc, in_maps, core_ids=list(range(nb)), trace=trace)
    out = np.stack([res.results[i]["out"] for i in range(nb)], axis=0)
    if trace:
        return out, res
    return out

